# revision 1
# baseline (speedup 1.0000x reference)
"""IoU metric kernel for Trainium2 (Bass/Tile), 8-core data-parallel over batch.

Problem: input [16,21,512,512] f32 logits, target [16,21,512,512] f32 0/1 masks.
  pred = argmax_C(input); per-(b,c): inter = sum(target * onehot(pred)),
  gt = sum(target), pr = sum(onehot(pred)); present = any(target) = (gt > 0).
  scores[c] = (sum_b present*inter) / (sum_b present*(gt+pr) - inter_s + eps) * counts
Returns (scores[1:], counts[1:]).

Sharding: batch 16 -> 8 cores x 2 images. Each core computes per-image [C,3]
partials (inter, gt, pr); host does the trivial cross-batch combine.

Per-core kernel layout: image pixel plane [512,512] split into chunks of 128
h-rows: tile [128 part, 21 classes, 512 w]. Engines:
  DVE : 20-op running-max chain + 21 tensor_tensor(is_equal) ops producing
        the one-hot (bf16 - exact for 0/1).
  POOL: one big prod = oh * t multiply (bf16) - offloads DVE; runs
        concurrently with the pr/gt matmul pass (separate prod tile).
  PE  : per-class selector-matmuls (E_c.T @ rhs adds colsum into PSUM row c)
        accumulating inter/gt/pr into three PSUM banks across chunks.
  DMA : HWDGE (nc.sync) big coalesced loads; target pre-cast to bf16 on the
        host (exact for 0/1 masks, halves target HBM traffic).
Known-good/bad: tensor_tensor_reduce and SWDGE cast-DMA were tried and
rejected (TTR crashes real HW despite passing CoreSim; SWDGE descriptor
generation on Q7 serializes with the POOL multiply).
"""

import os
import threading
from contextlib import ExitStack

import numpy as np

import concourse.bacc as bacc
import concourse.bass as bass
import concourse.mybir as mybir
import concourse.tile as tile
from concourse.alu_op_type import AluOpType
from concourse.bass_utils import run_bass_kernel_spmd

F32 = mybir.dt.float32
BF16 = mybir.dt.bfloat16

B, C, H, W = 16, 21, 512, 512
NCORES = 8
BPC = B // NCORES  # images per core
P = 128

# Tunables
USE_POOL_MUL = os.environ.get("IOU_POOL_MUL", "1") == "1"
T_BF16 = os.environ.get("IOU_T_BF16", "1") == "1"
INPLACE_MUL = os.environ.get("IOU_INPLACE_MUL", "0") == "1"


def build_kernel_ir(nc, bpc=BPC, n_classes=C, h=H, w=W):
    """Emit the Tile IR for one core's shard [bpc, n_classes, h, w]."""
    f = w  # free dim per chunk = image width
    chunks = h // P  # chunks per image (h rows of 128)

    t_dram_dt = BF16 if T_BF16 else F32
    inp = nc.dram_tensor("input", [bpc, n_classes, h, w], F32, kind="ExternalInput")
    tgt = nc.dram_tensor("target", [bpc, n_classes, h, w], t_dram_dt, kind="ExternalInput")
    stats = nc.dram_tensor("stats", [bpc, n_classes, 4], F32, kind="ExternalOutput")

    # [b, c, (j p), w] -> [b, j, p, c, w]
    inp_r = inp.ap().rearrange("b c (j p) w -> b j p c w", p=P)
    tgt_r = tgt.ap().rearrange("b c (j p) w -> b j p c w", p=P)
    stats_ap = stats.ap()

    t_dt = BF16 if T_BF16 else F32

    with tile.TileContext(nc) as tc, ExitStack() as ctx:
        data_pool = ctx.enter_context(tc.tile_pool(name="data", bufs=2))
        acc_pool = ctx.enter_context(tc.tile_pool(name="acc", bufs=1))
        out_pool = ctx.enter_context(tc.tile_pool(name="outp", bufs=1))
        psum_pool = ctx.enter_context(tc.tile_pool(name="psum", bufs=1, space="PSUM"))

        # Per-class selector weights: E[:, c, :] is [128, C] with column c all
        # ones -> matmul(E_c.T @ rhs) adds colsum(rhs) into PSUM row c only.
        sel_dt = BF16 if T_BF16 else F32
        sel = acc_pool.tile([P, n_classes, n_classes], sel_dt, tag="sel")
        nc.vector.memset(sel, 0.0)
        for c in range(n_classes):
            nc.vector.memset(sel[:, c, c : c + 1], 1.0)

        for img in range(bpc):
            psum_inter = psum_pool.tile([n_classes, f], F32, tag=f"pi{img}")
            psum_gt = psum_pool.tile([n_classes, f], F32, tag=f"pg{img}")
            psum_pr = psum_pool.tile([n_classes, f], F32, tag=f"pp{img}")

            for j in range(chunks):
                xb = data_pool.tile([P, n_classes, f], F32, tag="xb")
                nc.sync.dma_start(out=xb[:], in_=inp_r[img, j])
                tb = data_pool.tile([P, n_classes, f], t_dt, tag="tb")
                nc.sync.dma_start(out=tb[:], in_=tgt_r[img, j])

                # running max over classes, split DVE / POOL as two subtrees
                pm = int(os.environ.get("IOU_POOL_MAX", "0"))
                split = n_classes - pm if pm >= 2 else n_classes
                m = data_pool.tile([P, f], F32, tag="m")
                nc.vector.tensor_max(m[:], xb[:, 0, :], xb[:, 1, :])
                for c in range(2, split):
                    nc.vector.tensor_max(m[:], m[:], xb[:, c, :])
                if split < n_classes:
                    mp = data_pool.tile([P, f], F32, tag="mp")
                    nc.gpsimd.tensor_max(mp[:], xb[:, split, :], xb[:, split + 1, :])
                    for c in range(split + 2, n_classes):
                        nc.gpsimd.tensor_max(mp[:], mp[:], xb[:, c, :])
                    nc.vector.tensor_max(m[:], m[:], mp[:])

                # one-hot via is_equal vs the max
                oh_dt = BF16 if T_BF16 else F32
                oh = data_pool.tile(
                    [P, n_classes, f], oh_dt, tag="oh",
                    bufs=(2 if INPLACE_MUL else 1),
                )
                for c in range(n_classes):
                    nc.vector.tensor_tensor(
                        oh[:, c, :], xb[:, c, :], m[:], AluOpType.is_equal
                    )

                # pr matmuls must read oh before the (possibly in-place) mul
                for c in range(n_classes):
                    first = j == 0 and c == 0
                    last = j == chunks - 1 and c == n_classes - 1
                    nc.tensor.matmul(
                        psum_pr[:, :], sel[:, c, :], oh[:, c, :],
                        start=first, stop=last,
                    )
                    nc.tensor.matmul(
                        psum_gt[:, :], sel[:, c, :], tb[:, c, :],
                        start=first, stop=last,
                    )

                # prod = oh * t
                if INPLACE_MUL:
                    prod = oh
                else:
                    prod = data_pool.tile([P, n_classes, f], oh_dt, tag="prod", bufs=1)
                mul_eng = nc.gpsimd if USE_POOL_MUL else nc.vector
                if os.environ.get("IOU_SPLIT_MUL", "1") == "1":
                    half = n_classes // 2
                    mul_eng.tensor_mul(
                        prod[:, :half, :], oh[:, :half, :], tb[:, :half, :]
                    )
                    mul_eng.tensor_mul(
                        prod[:, half:, :], oh[:, half:, :], tb[:, half:, :]
                    )
                else:
                    mul_eng.tensor_mul(prod[:], oh[:], tb[:])

                for c in range(n_classes):
                    first = j == 0 and c == 0
                    last = j == chunks - 1 and c == n_classes - 1
                    nc.tensor.matmul(
                        psum_inter[:, :], sel[:, c, :], prod[:, c, :],
                        start=first, stop=last,
                    )

            # finalize image: [C,f] psum -> [C,1]; pr partition-reduce via PE
            res = out_pool.tile([n_classes, 4], F32, tag=f"res{img}")
            nc.vector.tensor_reduce(
                out=res[:, 0:1], in_=psum_inter[:], axis=mybir.AxisListType.X,
                op=AluOpType.add,
            )
            nc.vector.tensor_reduce(
                out=res[:, 1:2], in_=psum_gt[:], axis=mybir.AxisListType.X,
                op=AluOpType.add,
            )
            nc.vector.tensor_reduce(
                out=res[:, 2:3], in_=psum_pr[:], axis=mybir.AxisListType.X,
                op=AluOpType.add,
            )
            nc.vector.memset(res[:, 3:4], 0.0)
            nc.sync.dma_start(out=stats_ap[img], in_=res[:])

    return nc


_BUILD_LOCK = threading.Lock()
_NC_CACHE = {}


def get_compiled_nc(key="full"):
    with _BUILD_LOCK:
        if key not in _NC_CACHE:
            nc = bacc.Bacc("TRN2", target_bir_lowering=False, debug=False)
            build_kernel_ir(nc)
            nc.compile()
            _NC_CACHE[key] = nc
        return _NC_CACHE[key]


def combine_stats(stats_all):
    """stats_all: [B, C, >=3] per-image partials -> (scores[1:], counts[1:])."""
    stats_all = np.asarray(stats_all, dtype=np.float64)
    inter_bc = stats_all[..., 0]
    gt_bc = stats_all[..., 1]
    pr_bc = stats_all[..., 2]
    present = (gt_bc > 0).astype(np.float64)
    inter_s = (present * inter_bc).sum(0)
    union_s = (present * (gt_bc + pr_bc)).sum(0) - inter_s + 1e-7
    counts = present.sum(0)
    scores = (inter_s / union_s) * counts
    return (
        scores[1:].astype(np.float32),
        counts[1:].astype(np.float32),
    )


def kernel(input, target):
    import ml_dtypes

    inp = np.ascontiguousarray(np.asarray(input, dtype=np.float32))
    tgt = np.ascontiguousarray(np.asarray(target, dtype=np.float32))
    assert inp.shape == (B, C, H, W), inp.shape
    if T_BF16:
        # 0/1 masks are exact in bf16; halves target HBM traffic on-device
        tgt = tgt.astype(ml_dtypes.bfloat16)

    nc = get_compiled_nc()
    in_maps = [
        {
            "input": inp[i * BPC : (i + 1) * BPC],
            "target": tgt[i * BPC : (i + 1) * BPC],
        }
        for i in range(NCORES)
    ]
    res = run_bass_kernel_spmd(nc, in_maps, core_ids=list(range(NCORES)))
    stats_all = np.concatenate([r["stats"] for r in res.results], axis=0)  # [B,C,4]
    return combine_stats(stats_all)


if __name__ == "__main__":
    rng = np.random.default_rng(0)
    x = rng.standard_normal((B, C, H, W), dtype=np.float32)
    t = (rng.random((B, C, H, W)) < 0.05).astype(np.float32)
    s, c = kernel(input=x, target=t)
    print("scores:", s)
    print("counts:", c)



# revision 8
# speedup vs baseline: 2.3991x; 2.3991x over previous
"""IoU metric kernel v5 (hybrid LSB) for Trainium2, 8-core data-parallel.

Host packs target presence into the fp16 logit LSB:
    z = (fp16(x) & 0xFFFE) | t        (value noise ~1 ulp, same class as the
                                       fp16 cast noise; rel err ~2e-3)
Device per chunk [128, 21, 512]:
    m  = max_c z   (DVE pairwise tree, fp16 2x)
    mo = m | 1     (u16 bitcast tensor_scalar, 4x)
    oh = (z == m)  -> pr-mask, fp16 2x
    inter-mask, split by class:
      A classes (first NA): f = (z == mo)  on DVE at 2x. Bit logic: z_c==m|1
        iff c is the argmax AND its t-bit is 1 (if m is even, m|1 is 1 ulp
        ABOVE the max, which no z can equal -> no false positives).
      B classes (rest): prod = oh * tq on Pool (mult is one of the only TT
        ops the Pool engine supports on real TRN2).
    PE: gt (tq fp8 DoubleRow), pr (oh fp16 plain), inter (fA fp16 plain +
    prodB fp8 DR) column-sum passes into [32,512] psums (dual-fp8 ldweights
    needs 32-multiple columns), accumulated over the image's 4 chunks.
Separate fp8 target stream tq is still loaded for gt and the B-class prods.
"""

import os
import threading
from contextlib import ExitStack

import numpy as np

import concourse.bacc as bacc
import concourse.bass as bass
import concourse.mybir as mybir
import concourse.tile as tile
from concourse.alu_op_type import AluOpType
from concourse.bass_utils import run_bass_kernel_spmd

F32 = mybir.dt.float32
F16 = mybir.dt.float16
F8 = mybir.dt.float8e4
U16 = mybir.dt.uint16
DR = mybir.MatmulPerfMode.DoubleRow

B, C, H, W = 16, 21, 512, 512
NCORES = 8
BPC = B // NCORES
P = 128
NDRC = 32

NA = int(os.environ.get("IOU_NA", "8"))  # classes on the DVE LSB-eq route (even)
NA_TAIL = int(os.environ.get("IOU_NA_TAIL", "5"))  # extra DVE classes, final chunk
WSPLIT = int(os.environ.get("IOU_WSPLIT", "2"))


def build_kernel_ir(nc, bpc=BPC, n_classes=C, h=H, w=W):
    f = w
    chunks = h // P

    inp = nc.dram_tensor("input", [bpc, n_classes, h, w], F16, kind="ExternalInput")
    tgt = nc.dram_tensor("target", [bpc, n_classes, h, w], F8, kind="ExternalInput")
    stats = nc.dram_tensor("stats", [bpc, n_classes, 4], F32, kind="ExternalOutput")

    inp_r = inp.ap().rearrange("b c (j p) w -> b j p c w", p=P)
    tgt_r = tgt.ap().rearrange("b c (j p) w -> b j p c w", p=P)
    stats_ap = stats.ap()

    na = NA
    nb = n_classes - na
    npairs = n_classes // 2
    ws = f // WSPLIT

    with tile.TileContext(nc) as tc, ExitStack() as ctx:
        data_pool = ctx.enter_context(tc.tile_pool(name="data", bufs=2))
        acc_pool = ctx.enter_context(tc.tile_pool(name="acc", bufs=1))
        psum_pool = ctx.enter_context(tc.tile_pool(name="psum", bufs=1, space="PSUM"))

        sel16 = acc_pool.tile([P, n_classes, n_classes], F16, tag="sel16")
        nc.vector.memset(sel16, 0.0)
        for c in range(n_classes):
            nc.vector.memset(sel16[:, c, c : c + 1], 1.0)
        sel8 = acc_pool.tile([P, n_classes, n_classes], F8, tag="sel8")
        nc.vector.memset(sel8, 0.0)
        for c in range(n_classes):
            nc.vector.memset(sel8[:, c, c : c + 1], 1.0)
        sel_dr = []
        for i in range(npairs):
            t = acc_pool.tile([P, 2, NDRC], F8, tag=f"sel_dr{i}")
            nc.vector.memset(t, 0.0)
            nc.vector.memset(t[:, 0, 2 * i : 2 * i + 1], 1.0)
            nc.vector.memset(t[:, 1, 2 * i + 1 : 2 * i + 2], 1.0)
            sel_dr.append(t)

        def range_pass(ps, rhs_tile, base, lo, hi, sl, first_op, last_op):
            """Column-sum classes [lo,hi) of rhs_tile (rhs index = class-base)
            into psum rows lo..hi-1: DR for even-aligned pairs, plain fp8 for
            odd boundary classes."""
            ops = []
            c = lo
            if c % 2 and c < hi:
                ops.append(("plain", c)); c += 1
            while c + 1 < hi:
                ops.append(("dr", c)); c += 2
            if c < hi:
                ops.append(("plain", c))
            for idx, (kind, c) in enumerate(ops):
                st = first_op and idx == 0
                sp = last_op and idx == len(ops) - 1
                if kind == "dr":
                    nc.tensor.matmul(
                        ps[:, sl], sel_dr[c // 2][:, :, :],
                        rhs_tile[:, c - base : c - base + 2, sl],
                        start=st, stop=sp, perf_mode=DR,
                    )
                else:
                    nc.tensor.matmul(
                        ps[0:n_classes, sl], sel8[:, c, :],
                        rhs_tile[:, c - base, sl],
                        start=st, stop=sp,
                    )

        def gt_pass(ps, tb_tile, first, last):
            range_pass(ps, tb_tile, 1, 1, n_classes, slice(0, f), first, last)

        for img in range(bpc):
            psg = psum_pool.tile([NDRC, f], F32, tag=f"pg{img}")
            psp = psum_pool.tile([NDRC, f], F32, tag=f"pp{img}")
            psi = psum_pool.tile([NDRC, f], F32, tag=f"pi{img}")

            for j in range(chunks):
                first = j == 0
                last = j == chunks - 1

                very_first = img == 0 and j == 0
                xb = data_pool.tile([P, n_classes, f], F16, tag="xb")
                if very_first:
                    # split the very first load so the max tree starts ~4us
                    # earlier (per-(p,c) runs stay 512B -> no DMA penalty)
                    nc.sync.dma_start(
                        out=xb[:, :, 0 : f // 2], in_=inp_r[img, j][:, :, 0 : f // 2]
                    )
                    nc.sync.dma_start(
                        out=xb[:, :, f // 2 :], in_=inp_r[img, j][:, :, f // 2 :]
                    )
                else:
                    nc.sync.dma_start(out=xb[:], in_=inp_r[img, j])
                # class-0 stats are never read by the host (scores[1:]),
                # so skip its target bytes, one-hot, prod and psum columns.
                tb = data_pool.tile([P, n_classes - 1, f], F8, tag="tb")
                nc.sync.dma_start(out=tb[:], in_=tgt_r[img, j][:, 1:n_classes])

                # gt pass straight off tb (classes 1..20), whole W
                gt_pass(psg, tb, first, last)

                # max tree (W-halved on the very first chunk)
                t10 = data_pool.tile([P, 10, f], F16, tag="t10", bufs=1)
                t5 = data_pool.tile([P, 5, f], F16, tag="t5", bufs=1)
                t2 = data_pool.tile([P, 2, f], F16, tag="t2", bufs=1)
                m = data_pool.tile([P, f], F16, tag="m", bufs=1)
                mo = data_pool.tile([P, f], F16, tag="mo", bufs=1)
                for hsl in ([slice(0, f // 2), slice(f // 2, f)] if very_first
                            else [slice(0, f)]):
                    nc.vector.tensor_max(
                        t10[:, :, hsl], xb[:, 0:10, hsl], xb[:, 10:20, hsl]
                    )
                    nc.vector.tensor_max(
                        t5[:, :, hsl], t10[:, 0:5, hsl], t10[:, 5:10, hsl]
                    )
                    nc.vector.tensor_max(
                        t2[:, :, hsl], t5[:, 0:2, hsl], t5[:, 2:4, hsl]
                    )
                    nc.vector.tensor_max(m[:, hsl], t2[:, 0, hsl], t2[:, 1, hsl])
                    nc.vector.tensor_max(m[:, hsl], m[:, hsl], t5[:, 4, hsl])
                    nc.vector.tensor_max(m[:, hsl], m[:, hsl], xb[:, 20, hsl])
                    # mo = m | 1 (u16 bitcast views)
                    nc.vector.tensor_scalar(
                        mo[:, hsl].bitcast(U16), m[:, hsl].bitcast(U16), 1, None,
                        AluOpType.bitwise_or,
                    )

                # final chunk leans harder on DVE so the Pool op doesn't
                # serialize after the last eq. Classes covered: 1..20.
                # A = [1, 1+na_j), B = [1+na_j, 21).
                extra = NA_TAIL if (img == bpc - 1 and last) else 0
                na_j = na + extra
                bs_j = 1 + na_j          # B range start (class id)
                nb_j = n_classes - bs_j  # B class count

                oh = data_pool.tile([P, n_classes - 1, f], F16, tag="oh")
                fA = data_pool.tile([P, na + NA_TAIL, f], F16, tag="fA")
                prodB = data_pool.tile([P, nb, f], F8, tag="prodB")
                for si in range(WSPLIT):
                    sl = slice(si * ws, (si + 1) * ws)
                    mb = (
                        m[:, sl]
                        .rearrange("p (o w) -> p o w", o=1)
                        .broadcast_to([P, n_classes - 1, ws])
                    )
                    nc.vector.tensor_tensor(
                        oh[:, :, sl], xb[:, 1:, sl], mb, AluOpType.is_equal
                    )
                    # B-class prods on Pool (reads oh slice as it lands);
                    # oh/tb index = class - 1
                    nc.gpsimd.tensor_tensor(
                        prodB[:, 0:nb_j, sl], oh[:, bs_j - 1 :, sl],
                        tb[:, bs_j - 1 :, sl], AluOpType.mult,
                    )
                    # A-class inter-masks on DVE at 2x (classes 1..na_j)
                    mob = (
                        mo[:, sl]
                        .rearrange("p (o w) -> p o w", o=1)
                        .broadcast_to([P, na_j, ws])
                    )
                    nc.vector.tensor_tensor(
                        fA[:, 0:na_j, sl], xb[:, 1 : 1 + na_j, sl], mob,
                        AluOpType.is_equal,
                    )
                    # pr pass (plain fp16 per W slice, classes 1..20)
                    for c in range(1, n_classes):
                        nc.tensor.matmul(
                            psp[0:n_classes, sl], sel16[:, c, :],
                            oh[:, c - 1, sl],
                            start=first and si == 0 and c == 1,
                            stop=last and si == WSPLIT - 1
                            and c == n_classes - 1,
                        )
                    # inter pass: A plain fp16 + B range (DR + boundary plains)
                    for c in range(1, 1 + na_j):
                        nc.tensor.matmul(
                            psi[0:n_classes, sl], sel16[:, c, :],
                            fA[:, c - 1, sl],
                            start=first and si == 0 and c == 1, stop=False,
                        )
                    range_pass(
                        psi, prodB, bs_j, bs_j, n_classes, sl,
                        first_op=False,
                        last_op=last and si == WSPLIT - 1,
                    )

            res = acc_pool.tile([n_classes, 4], F32, tag=f"res{img}")
            junk = acc_pool.tile([n_classes, f], F32, tag="junk")
            nc.scalar.activation(
                junk[:], psg[0:n_classes, :], mybir.ActivationFunctionType.Copy,
                accum_out=res[:, 0:1],
            )
            nc.scalar.activation(
                junk[:], psp[0:n_classes, :], mybir.ActivationFunctionType.Copy,
                accum_out=res[:, 1:2],
            )
            nc.scalar.activation(
                junk[:], psi[0:n_classes, :], mybir.ActivationFunctionType.Copy,
                accum_out=res[:, 2:3],
            )
            nc.vector.memset(res[:, 3:4], 0.0)
            nc.sync.dma_start(out=stats_ap[img], in_=res[:])

    return nc


_BUILD_LOCK = threading.Lock()
_NC_CACHE = {}


def get_compiled_nc(key="full"):
    with _BUILD_LOCK:
        if key not in _NC_CACHE:
            nc = bacc.Bacc("TRN2", target_bir_lowering=False, debug=False)
            build_kernel_ir(nc)
            nc.compile()
            _NC_CACHE[key] = nc
        return _NC_CACHE[key]


def combine_stats(stats_all):
    """stats_all: [B, C, >=3] rows (gt, pr, inter) -> (scores[1:], counts[1:])."""
    stats_all = np.asarray(stats_all, dtype=np.float64)
    gt_bc = stats_all[..., 0]
    pr_bc = stats_all[..., 1]
    inter_bc = stats_all[..., 2]
    present = (gt_bc > 0).astype(np.float64)
    inter_s = (present * inter_bc).sum(0)
    union_s = (present * (gt_bc + pr_bc)).sum(0) - inter_s + 1e-7
    counts = present.sum(0)
    scores = (inter_s / union_s) * counts
    return (
        scores[1:].astype(np.float32),
        counts[1:].astype(np.float32),
    )


def pack_inputs(input, target):
    import ml_dtypes

    x16 = np.asarray(input, dtype=np.float32).astype(np.float16)
    tbit = (np.asarray(target) != 0)
    z = ((x16.view(np.uint16) & np.uint16(0xFFFE))
         | tbit.astype(np.uint16)).view(np.float16)
    tq = tbit.astype(ml_dtypes.float8_e4m3)
    return z, tq


def kernel(input, target):
    z, tq = pack_inputs(input, target)
    assert z.shape == (B, C, H, W), z.shape

    nc = get_compiled_nc()
    in_maps = [
        {
            "input": np.ascontiguousarray(z[i * BPC : (i + 1) * BPC]),
            "target": np.ascontiguousarray(tq[i * BPC : (i + 1) * BPC]),
        }
        for i in range(NCORES)
    ]
    res = run_bass_kernel_spmd(nc, in_maps, core_ids=list(range(NCORES)))
    stats_all = np.concatenate([r["stats"] for r in res.results], axis=0)
    return combine_stats(stats_all)


if __name__ == "__main__":
    rng = np.random.default_rng(0)
    x = rng.standard_normal((B, C, H, W), dtype=np.float32)
    t = (rng.random((B, C, H, W)) < 0.05).astype(np.float32)
    s, c = kernel(input=x, target=t)
    print("scores:", s)
    print("counts:", c)


# revision 11
# speedup vs baseline: 2.4243x; 1.0105x over previous
"""IoU metric kernel v5 (hybrid LSB) for Trainium2, 8-core data-parallel.

Host packs target presence into the fp16 logit LSB:
    z = (fp16(x) & 0xFFFE) | t        (value noise ~1 ulp, same class as the
                                       fp16 cast noise; rel err ~2e-3)
Device per chunk [128, 21, 512]:
    m  = max_c z   (DVE pairwise tree, fp16 2x)
    mo = m | 1     (u16 bitcast tensor_scalar, 4x)
    oh = (z == m)  -> pr-mask, fp16 2x
    inter-mask, split by class:
      A classes (first NA): f = (z == mo)  on DVE at 2x. Bit logic: z_c==m|1
        iff c is the argmax AND its t-bit is 1 (if m is even, m|1 is 1 ulp
        ABOVE the max, which no z can equal -> no false positives).
      B classes (rest): prod = oh * tq on Pool (mult is one of the only TT
        ops the Pool engine supports on real TRN2).
    PE: gt (tq fp8 DoubleRow), pr (oh fp16 plain), inter (fA fp16 plain +
    prodB fp8 DR) column-sum passes into [32,512] psums (dual-fp8 ldweights
    needs 32-multiple columns), accumulated over the image's 4 chunks.
Separate fp8 target stream tq is still loaded for gt and the B-class prods.
"""

import os
import threading
from contextlib import ExitStack

import numpy as np

import concourse.bacc as bacc
import concourse.bass as bass
import concourse.mybir as mybir
import concourse.tile as tile
from concourse.alu_op_type import AluOpType
from concourse.bass_utils import run_bass_kernel_spmd

F32 = mybir.dt.float32
F16 = mybir.dt.float16
F8 = mybir.dt.float8e4
U16 = mybir.dt.uint16
DR = mybir.MatmulPerfMode.DoubleRow

B, C, H, W = 16, 21, 512, 512
NCORES = 8
BPC = B // NCORES
P = 128
NDRC = 32

NA = int(os.environ.get("IOU_NA", "8"))  # classes on the DVE LSB-eq route (even)
NA_TAIL = int(os.environ.get("IOU_NA_TAIL", "5"))
NA_FIRST = int(os.environ.get("IOU_NA_FIRST", "3"))  # lighter DVE on early chunks
NF_CHUNKS = int(os.environ.get("IOU_NF_CHUNKS", "2"))  # extra DVE classes, final chunk
WSPLIT = int(os.environ.get("IOU_WSPLIT", "2"))


def build_kernel_ir(nc, bpc=BPC, n_classes=C, h=H, w=W):
    f = w
    chunks = h // P

    inp = nc.dram_tensor("input", [bpc, n_classes, h, w], F16, kind="ExternalInput")
    tgt = nc.dram_tensor("target", [bpc, n_classes, h, w], F8, kind="ExternalInput")
    stats = nc.dram_tensor("stats", [bpc, n_classes, 4], F32, kind="ExternalOutput")

    inp_r = inp.ap().rearrange("b c (j p) w -> b j p c w", p=P)
    tgt_r = tgt.ap().rearrange("b c (j p) w -> b j p c w", p=P)
    stats_ap = stats.ap()

    na = NA
    nb = n_classes - na
    npairs = n_classes // 2
    ws = f // WSPLIT

    with tile.TileContext(nc) as tc, ExitStack() as ctx:
        data_pool = ctx.enter_context(tc.tile_pool(name="data", bufs=2))
        acc_pool = ctx.enter_context(tc.tile_pool(name="acc", bufs=1))
        psum_pool = ctx.enter_context(tc.tile_pool(name="psum", bufs=1, space="PSUM"))

        sel16 = acc_pool.tile([P, n_classes, n_classes], F16, tag="sel16")
        nc.vector.memset(sel16, 0.0)
        for c in range(n_classes):
            nc.vector.memset(sel16[:, c, c : c + 1], 1.0)
        sel8 = acc_pool.tile([P, n_classes, n_classes], F8, tag="sel8")
        nc.vector.memset(sel8, 0.0)
        for c in range(n_classes):
            nc.vector.memset(sel8[:, c, c : c + 1], 1.0)
        sel_dr = []
        for i in range(npairs):
            t = acc_pool.tile([P, 2, NDRC], F8, tag=f"sel_dr{i}")
            nc.vector.memset(t, 0.0)
            nc.vector.memset(t[:, 0, 2 * i : 2 * i + 1], 1.0)
            nc.vector.memset(t[:, 1, 2 * i + 1 : 2 * i + 2], 1.0)
            sel_dr.append(t)

        def range_pass(ps, rhs_tile, base, lo, hi, sl, first_op, last_op):
            """Column-sum classes [lo,hi) of rhs_tile (rhs index = class-base)
            into psum rows lo..hi-1: DR for even-aligned pairs, plain fp8 for
            odd boundary classes."""
            ops = []
            c = lo
            if c % 2 and c < hi:
                ops.append(("plain", c)); c += 1
            while c + 1 < hi:
                ops.append(("dr", c)); c += 2
            if c < hi:
                ops.append(("plain", c))
            for idx, (kind, c) in enumerate(ops):
                st = first_op and idx == 0
                sp = last_op and idx == len(ops) - 1
                if kind == "dr":
                    nc.tensor.matmul(
                        ps[:, sl], sel_dr[c // 2][:, :, :],
                        rhs_tile[:, c - base : c - base + 2, sl],
                        start=st, stop=sp, perf_mode=DR,
                    )
                else:
                    nc.tensor.matmul(
                        ps[0:n_classes, sl], sel8[:, c, :],
                        rhs_tile[:, c - base, sl],
                        start=st, stop=sp,
                    )

        def gt_pass(ps, tb_tile, first, last):
            range_pass(ps, tb_tile, 1, 1, n_classes, slice(0, f), first, last)

        for img in range(bpc):
            psg = psum_pool.tile([NDRC, f], F32, tag=f"pg{img}")
            psp = psum_pool.tile([NDRC, f], F32, tag=f"pp{img}")
            psi = psum_pool.tile([NDRC, f], F32, tag=f"pi{img}")

            for j in range(chunks):
                first = j == 0
                last = j == chunks - 1

                very_first = img == 0 and j == 0
                xb = data_pool.tile([P, n_classes, f], F16, tag="xb")
                if very_first:
                    # split the very first load so the max tree starts ~4us
                    # earlier (per-(p,c) runs stay 512B -> no DMA penalty)
                    nc.sync.dma_start(
                        out=xb[:, :, 0 : f // 2], in_=inp_r[img, j][:, :, 0 : f // 2]
                    )
                    nc.sync.dma_start(
                        out=xb[:, :, f // 2 :], in_=inp_r[img, j][:, :, f // 2 :]
                    )
                else:
                    nc.sync.dma_start(out=xb[:], in_=inp_r[img, j])
                # class-0 stats are never read by the host (scores[1:]),
                # so skip its target bytes, one-hot, prod and psum columns.
                tb = data_pool.tile([P, n_classes - 1, f], F8, tag="tb")
                nc.sync.dma_start(out=tb[:], in_=tgt_r[img, j][:, 1:n_classes])

                # gt pass straight off tb (classes 1..20), whole W
                gt_pass(psg, tb, first, last)

                # max tree (W-halved on the very first chunk)
                t10 = data_pool.tile([P, 10, f], F16, tag="t10", bufs=1)
                t5 = data_pool.tile([P, 5, f], F16, tag="t5", bufs=1)
                t2 = data_pool.tile([P, 2, f], F16, tag="t2", bufs=1)
                m = data_pool.tile([P, f], F16, tag="m", bufs=1)
                mo = data_pool.tile([P, f], F16, tag="mo", bufs=1)
                for hsl in ([slice(0, f // 2), slice(f // 2, f)] if very_first
                            else [slice(0, f)]):
                    nc.vector.tensor_max(
                        t10[:, :, hsl], xb[:, 0:10, hsl], xb[:, 10:20, hsl]
                    )
                    nc.vector.tensor_max(
                        t5[:, :, hsl], t10[:, 0:5, hsl], t10[:, 5:10, hsl]
                    )
                    nc.vector.tensor_max(
                        t2[:, :, hsl], t5[:, 0:2, hsl], t5[:, 2:4, hsl]
                    )
                    nc.vector.tensor_max(m[:, hsl], t2[:, 0, hsl], t2[:, 1, hsl])
                    nc.vector.tensor_max(m[:, hsl], m[:, hsl], t5[:, 4, hsl])
                    nc.vector.tensor_max(m[:, hsl], m[:, hsl], xb[:, 20, hsl])
                    # mo = m | 1 (u16 bitcast views)
                    nc.vector.tensor_scalar(
                        mo[:, hsl].bitcast(U16), m[:, hsl].bitcast(U16), 1, None,
                        AluOpType.bitwise_or,
                    )

                # final chunk leans harder on DVE so the Pool op doesn't
                # serialize after the last eq. Classes covered: 1..20.
                # A = [1, 1+na_j), B = [1+na_j, 21).
                extra = NA_TAIL if (img == bpc - 1 and last) else 0
                if img == 0 and j < NF_CHUNKS:
                    extra = NA_FIRST - na  # early chunks lean on Pool instead
                na_j = na + extra
                bs_j = 1 + na_j          # B range start (class id)
                nb_j = n_classes - bs_j  # B class count

                oh = data_pool.tile([P, n_classes - 1, f], F16, tag="oh")
                fA = data_pool.tile([P, na + NA_TAIL, f], F16, tag="fA")
                prodB = data_pool.tile([P, n_classes - 1 - min(na, NA_FIRST), f], F8, tag="prodB")
                for si in range(WSPLIT):
                    sl = slice(si * ws, (si + 1) * ws)
                    mb = (
                        m[:, sl]
                        .rearrange("p (o w) -> p o w", o=1)
                        .broadcast_to([P, n_classes - 1, ws])
                    )
                    nc.vector.tensor_tensor(
                        oh[:, :, sl], xb[:, 1:, sl], mb, AluOpType.is_equal
                    )
                    # B-class prods on Pool (reads oh slice as it lands);
                    # oh/tb index = class - 1
                    nc.gpsimd.tensor_tensor(
                        prodB[:, 0:nb_j, sl], oh[:, bs_j - 1 :, sl],
                        tb[:, bs_j - 1 :, sl], AluOpType.mult,
                    )
                    # A-class inter-masks on DVE at 2x (classes 1..na_j)
                    mob = (
                        mo[:, sl]
                        .rearrange("p (o w) -> p o w", o=1)
                        .broadcast_to([P, na_j, ws])
                    )
                    nc.vector.tensor_tensor(
                        fA[:, 0:na_j, sl], xb[:, 1 : 1 + na_j, sl], mob,
                        AluOpType.is_equal,
                    )
                    # pr pass (plain fp16 per W slice, classes 1..20)
                    for c in range(1, n_classes):
                        nc.tensor.matmul(
                            psp[0:n_classes, sl], sel16[:, c, :],
                            oh[:, c - 1, sl],
                            start=first and si == 0 and c == 1,
                            stop=last and si == WSPLIT - 1
                            and c == n_classes - 1,
                        )
                    # inter pass: A plain fp16 + B range (DR + boundary plains)
                    for c in range(1, 1 + na_j):
                        nc.tensor.matmul(
                            psi[0:n_classes, sl], sel16[:, c, :],
                            fA[:, c - 1, sl],
                            start=first and si == 0 and c == 1, stop=False,
                        )
                    range_pass(
                        psi, prodB, bs_j, bs_j, n_classes, sl,
                        first_op=False,
                        last_op=last and si == WSPLIT - 1,
                    )

            res = acc_pool.tile([n_classes, 4], F32, tag=f"res{img}")
            junk = acc_pool.tile([n_classes, f], F32, tag="junk")
            nc.scalar.activation(
                junk[:], psg[0:n_classes, :], mybir.ActivationFunctionType.Copy,
                accum_out=res[:, 0:1],
            )
            nc.scalar.activation(
                junk[:], psp[0:n_classes, :], mybir.ActivationFunctionType.Copy,
                accum_out=res[:, 1:2],
            )
            nc.scalar.activation(
                junk[:], psi[0:n_classes, :], mybir.ActivationFunctionType.Copy,
                accum_out=res[:, 2:3],
            )
            nc.vector.memset(res[:, 3:4], 0.0)
            nc.sync.dma_start(out=stats_ap[img], in_=res[:])

    return nc


_BUILD_LOCK = threading.Lock()
_NC_CACHE = {}


def get_compiled_nc(key="full"):
    with _BUILD_LOCK:
        if key not in _NC_CACHE:
            nc = bacc.Bacc("TRN2", target_bir_lowering=False, debug=False)
            build_kernel_ir(nc)
            nc.compile()
            _NC_CACHE[key] = nc
        return _NC_CACHE[key]


def combine_stats(stats_all):
    """stats_all: [B, C, >=3] rows (gt, pr, inter) -> (scores[1:], counts[1:])."""
    stats_all = np.asarray(stats_all, dtype=np.float64)
    gt_bc = stats_all[..., 0]
    pr_bc = stats_all[..., 1]
    inter_bc = stats_all[..., 2]
    present = (gt_bc > 0).astype(np.float64)
    inter_s = (present * inter_bc).sum(0)
    union_s = (present * (gt_bc + pr_bc)).sum(0) - inter_s + 1e-7
    counts = present.sum(0)
    scores = (inter_s / union_s) * counts
    return (
        scores[1:].astype(np.float32),
        counts[1:].astype(np.float32),
    )


def pack_inputs(input, target):
    import ml_dtypes

    x16 = np.asarray(input, dtype=np.float32).astype(np.float16)
    tbit = (np.asarray(target) != 0)
    z = ((x16.view(np.uint16) & np.uint16(0xFFFE))
         | tbit.astype(np.uint16)).view(np.float16)
    tq = tbit.astype(ml_dtypes.float8_e4m3)
    return z, tq


def kernel(input, target):
    z, tq = pack_inputs(input, target)
    assert z.shape == (B, C, H, W), z.shape

    nc = get_compiled_nc()
    in_maps = [
        {
            "input": np.ascontiguousarray(z[i * BPC : (i + 1) * BPC]),
            "target": np.ascontiguousarray(tq[i * BPC : (i + 1) * BPC]),
        }
        for i in range(NCORES)
    ]
    res = run_bass_kernel_spmd(nc, in_maps, core_ids=list(range(NCORES)))
    stats_all = np.concatenate([r["stats"] for r in res.results], axis=0)
    return combine_stats(stats_all)


if __name__ == "__main__":
    rng = np.random.default_rng(0)
    x = rng.standard_normal((B, C, H, W), dtype=np.float32)
    t = (rng.random((B, C, H, W)) < 0.05).astype(np.float32)
    s, c = kernel(input=x, target=t)
    print("scores:", s)
    print("counts:", c)
